# revision 4
# baseline (speedup 1.0000x reference)
"""MCANet channel-attention kernel for TRN2 (8 NeuronCores, data-parallel).

Reference math (the conv1x1+softmax branch in the module is dead code —
its result is deleted and never used):
    z[b,c]    = mean_{h,w} x[b,c,h,w]
    gate[b,c] = sigmoid(z[b,c] * w1d[c, center])       # center tap of the 1D conv
    out       = x * gate[:, :, None, None]

Per core: 2 batches of (512, 64*64) f32 — 16 MB in + 16 MB out, one pass,
so the kernel is HBM-bound. Channels map to SBUF partitions (4 blocks of
128), pixels to the free axis; per (batch, channel-block) tile: 2 MB load
on the SP HWDGE ring, DVE reduce, ACT sigmoid (center-tap weight fused in
as the activation scale AP), then 2x(in-place 1 MB gate-multiply + store)
on the ACT HWDGE ring.

Measured on HW (For_i slope method, see bench.py): ~103 us per per-core
workload, vs ~52 us load-only / ~54 us store-only microbenchmarks — i.e.
at the half-duplex HBM bound (~320 GB/s/core each way, no dynamic stack
sharing: per-core rate is flat whether 2, 4, or 8 cores are active).
Split rings matter: stores wait at the issuing sequencer for their tile's
multiply, so queueing them behind later loads on one ring (the previous
layout) head-of-line blocks the load stream (~112 us). Load chunking
below 2 MB, 4-way store chunking, single 2 MB stores, SWDGE store
offload, and 2-ring loads all measured neutral-to-worse.
"""

import numpy as np

import concourse.tile as tile
from concourse import bacc, mybir
from concourse.bass_utils import run_bass_kernel_spmd

B, C, H, W = 16, 512, 64, 64
HW = H * W
K_CENTER = 2  # (5 - 1) // 2
N_CORES = 8
B_PER = B // N_CORES  # 2
P = 128
CBLK = C // P  # 4

_NC_CACHE = {}


def _build_nc(repeats=1, loop_n=None):
    nc = bacc.Bacc("TRN2", debug=False, target_bir_lowering=False,
                   num_devices=N_CORES)
    x_in = nc.dram_tensor("x", [B_PER, C, HW], mybir.dt.float32,
                          kind="ExternalInput").ap()
    wc_in = nc.dram_tensor("wc", [C], mybir.dt.float32,
                           kind="ExternalInput").ap()
    out = nc.dram_tensor("out", [B_PER, C, HW], mybir.dt.float32,
                         kind="ExternalOutput").ap()

    # DMA ring split: loads issue on the SP ring (nc.sync), stores on the
    # ACT ring (nc.scalar). HWDGE descriptors drain FIFO per ring and the
    # issuing sequencer blocks on unmet deps, so a store waiting for its
    # tile's multiply must not queue ahead of later loads — on separate
    # rings loads free-run while stores trail the ACT multiplies with zero
    # cross-engine sync (mul and store issue back-to-back on ACT).
    with tile.TileContext(nc) as tc:
        with (
            tc.tile_pool(name="xp", bufs=B_PER * CBLK) as xp,
            tc.tile_pool(name="sp", bufs=8 * max(1, repeats)) as sp,
            tc.tile_pool(name="wp", bufs=1) as wp,
        ):
            # wc laid out [partition, block]: element [p, t] = wc[t*128 + p]
            wt = wp.tile([P, CBLK], mybir.dt.float32)
            nc.sync.dma_start(wt[:], wc_in.rearrange("(t p) -> p t", p=P))

            def body():
                for b in range(B_PER):
                    for t in range(CBLK):
                        xt = xp.tile([P, HW], mybir.dt.float32)
                        nc.sync.dma_start(xt[:], x_in[b, t * P:(t + 1) * P, :])

                        s = sp.tile([P, 1], mybir.dt.float32)
                        nc.vector.reduce_sum(s[:], xt[:],
                                             axis=mybir.AxisListType.X)
                        # gate = sigmoid(sum * (w_center/HW)), w as scale AP
                        g = sp.tile([P, 1], mybir.dt.float32)
                        nc.scalar.activation(g[:], s[:],
                                             mybir.ActivationFunctionType.Sigmoid,
                                             scale=wt[:, t:t + 1])
                        # xt *= gate in place on ScalarE, store each chunk
                        # right behind its multiply on the same engine.
                        half = HW // 2
                        for h in range(2):
                            sl = slice(h * half, (h + 1) * half)
                            nc.scalar.mul(xt[:, sl], xt[:, sl], g[:])
                            nc.scalar.dma_start(
                                out[b, t * P:(t + 1) * P, sl], xt[:, sl])

            if loop_n is not None:
                with tc.For_i(0, loop_n):
                    body()
            else:
                for _ in range(repeats):
                    body()
    # Legalizes sync waits (≤1 per instruction, extras hoisted onto
    # EventSemaphore instructions) among other lowering passes.
    nc.compile()
    return nc


def _get_nc():
    if "nc" not in _NC_CACHE:
        _NC_CACHE["nc"] = _build_nc()
    return _NC_CACHE["nc"]


def _run(x, w1d, trace=False):
    x = np.ascontiguousarray(np.asarray(x, dtype=np.float32)).reshape(B, C, HW)
    # Fold the mean's 1/HW into the center-tap weight: HW is a power of two,
    # so w/HW is exact and sum*(w/HW) rounds identically to (sum/HW)*w.
    wc = np.ascontiguousarray(
        np.asarray(w1d, dtype=np.float32)[:, K_CENTER] / float(HW))
    nc = _get_nc()
    in_maps = [{"x": x[i * B_PER:(i + 1) * B_PER], "wc": wc}
               for i in range(N_CORES)]
    res = run_bass_kernel_spmd(nc, in_maps, list(range(N_CORES)), trace=trace)
    out = np.concatenate([res.results[i]["out"] for i in range(N_CORES)],
                         axis=0)
    return out.reshape(B, C, H, W), res.exec_time_ns


def kernel(x, w1x1=None, b1x1=None, w1d=None):
    out, _ = _run(x, w1d)
    return out



# revision 8
# speedup vs baseline: 1.0175x; 1.0175x over previous
"""MCANet channel-attention kernel for TRN2 (8 NeuronCores, data-parallel).

Reference math (the conv1x1+softmax branch in the module is dead code —
its result is deleted and never used):
    z[b,c]    = mean_{h,w} x[b,c,h,w]
    gate[b,c] = sigmoid(z[b,c] * w1d[c, center])       # center tap of the 1D conv
    out       = x * gate[:, :, None, None]

Per core: 2 batches of (512, 64*64) f32 — 16 MB in + 16 MB out, one pass,
so the kernel is HBM-bound. Channels map to SBUF partitions (4 blocks of
128), pixels to the free axis; per (batch, channel-block) tile: 2 MB load
on the SP HWDGE ring, DVE reduce, ACT sigmoid (center-tap weight fused in
as the activation scale AP), then 2x(in-place 1 MB gate-multiply + store)
on the ACT HWDGE ring.

Measured on HW (For_i slope method, see bench.py): ~103 us per per-core
workload, vs ~52 us load-only / ~54 us store-only microbenchmarks — i.e.
at the half-duplex HBM bound (~320 GB/s/core each way, no dynamic stack
sharing: per-core rate is flat whether 2, 4, or 8 cores are active).
Split rings matter: stores wait at the issuing sequencer for their tile's
multiply, so queueing them behind later loads on one ring (the previous
layout) head-of-line blocks the load stream (~112 us). Load chunking
below 2 MB, 4-way store chunking, single 2 MB stores, SWDGE store
offload, and 2-ring loads all measured neutral-to-worse.
"""

import numpy as np

import concourse.tile as tile
from concourse import bacc, mybir
from concourse.bass_utils import run_bass_kernel_spmd

B, C, H, W = 16, 512, 64, 64
HW = H * W
K_CENTER = 2  # (5 - 1) // 2
N_CORES = 8
B_PER = B // N_CORES  # 2
P = 128
CBLK = C // P  # 4

_NC_CACHE = {}


def _build_nc(repeats=1, loop_n=None):
    nc = bacc.Bacc("TRN2", debug=False, target_bir_lowering=False,
                   num_devices=N_CORES)
    x_in = nc.dram_tensor("x", [B_PER, C, HW], mybir.dt.float32,
                          kind="ExternalInput").ap()
    wc_in = nc.dram_tensor("wc", [C], mybir.dt.float32,
                           kind="ExternalInput").ap()
    out = nc.dram_tensor("out", [B_PER, C, HW], mybir.dt.float32,
                         kind="ExternalOutput").ap()

    # DMA ring split: loads issue on the SP ring (nc.sync), stores on the
    # ACT ring (nc.scalar). HWDGE descriptors drain FIFO per ring and the
    # issuing sequencer blocks on unmet deps, so a store waiting for its
    # tile's multiply must not queue ahead of later loads — on separate
    # rings loads free-run while stores trail the ACT multiplies with zero
    # cross-engine sync (mul and store issue back-to-back on ACT).
    with tile.TileContext(nc) as tc:
        with (
            tc.tile_pool(name="xp", bufs=B_PER * CBLK) as xp,
            tc.tile_pool(name="sp", bufs=8 * max(1, repeats)) as sp,
            tc.tile_pool(name="wp", bufs=1) as wp,
        ):
            # wc laid out [partition, block]: element [p, t] = wc[t*128 + p]
            wt = wp.tile([P, CBLK], mybir.dt.float32)
            nc.sync.dma_start(wt[:], wc_in.rearrange("(t p) -> p t", p=P))

            def body():
                for b in range(B_PER):
                    for t in range(CBLK):
                        xt = xp.tile([P, HW], mybir.dt.float32)
                        nc.sync.dma_start(xt[:], x_in[b, t * P:(t + 1) * P, :])

                        s = sp.tile([P, 1], mybir.dt.float32)
                        nc.vector.reduce_sum(s[:], xt[:],
                                             axis=mybir.AxisListType.X)
                        # gate = sigmoid(sum * (w_center/HW)), w as scale AP
                        g = sp.tile([P, 1], mybir.dt.float32)
                        nc.scalar.activation(g[:], s[:],
                                             mybir.ActivationFunctionType.Sigmoid,
                                             scale=wt[:, t:t + 1])
                        # xt *= gate in place on ScalarE, store each chunk
                        # right behind its multiply on the same engine.
                        half = HW // 2
                        for h in range(2):
                            sl = slice(h * half, (h + 1) * half)
                            nc.scalar.mul(xt[:, sl], xt[:, sl], g[:])
                            nc.scalar.dma_start(
                                out[b, t * P:(t + 1) * P, sl], xt[:, sl])

            if loop_n is not None:
                with tc.For_i(0, loop_n):
                    body()
            else:
                for _ in range(repeats):
                    body()
    # Legalizes sync waits (≤1 per instruction, extras hoisted onto
    # EventSemaphore instructions) among other lowering passes.
    nc.compile()
    return nc


def _get_nc():
    if "nc" not in _NC_CACHE:
        _NC_CACHE["nc"] = _build_nc()
    return _NC_CACHE["nc"]


def _get_runner():
    """Persistent-jit SPMD dispatch for the compiled module.

    run_bass_kernel_spmd re-traces and re-jits the PJRT wrapper on every
    call (~20 s wall under axon). This mirrors its bass2jax lowering once
    and reuses the jitted executable, so repeat kernel() calls only pay
    input upload + device exec. Falls back to run_bass_kernel_spmd.
    """
    if "run" in _NC_CACHE:
        return _NC_CACHE["run"]
    import jax
    from jax.experimental.shard_map import shard_map
    from jax.sharding import Mesh, NamedSharding, PartitionSpec
    from concourse import bass2jax

    nc = _get_nc()
    bass2jax.install_neuronx_cc_hook()
    partition_name = (nc.partition_id_tensor.name
                      if nc.partition_id_tensor else None)
    in_names, out_names, out_avals = [], [], []
    for alloc in nc.m.functions[0].allocations:
        if not isinstance(alloc, mybir.MemoryLocationSet):
            continue
        name = alloc.memorylocations[0].name
        if alloc.kind == "ExternalInput":
            if name != partition_name:
                in_names.append(name)
        elif alloc.kind == "ExternalOutput":
            out_names.append(name)
            out_avals.append(jax.core.ShapedArray(
                tuple(alloc.tensor_shape), mybir.dt.np(alloc.dtype)))
    all_in_names = in_names + out_names
    if partition_name is not None:
        all_in_names.append(partition_name)

    def _body(*args):
        operands = list(args)
        if partition_name is not None:
            operands.append(bass2jax.partition_id_tensor())
        return tuple(bass2jax._bass_exec_p.bind(
            *operands,
            out_avals=tuple(out_avals),
            in_names=tuple(all_in_names),
            out_names=tuple(out_names),
            lowering_input_output_aliases=(),
            sim_require_finite=True,
            sim_require_nnan=True,
            nc=nc,
        ))

    mesh = Mesh(np.asarray(jax.devices()[:N_CORES]), ("core",))
    spec = PartitionSpec("core")
    sharding = NamedSharding(mesh, spec)
    n_args = len(in_names) + len(out_names)
    jitted = jax.jit(
        shard_map(_body, mesh=mesh, in_specs=(spec,) * n_args,
                  out_specs=(spec,) * len(out_names), check_rep=False),
        keep_unused=True,
    )
    assert in_names == ["x", "wc"] and out_names == ["out"], (
        in_names, out_names)

    # The zero output-seed operand never changes: upload it once and keep
    # it device-resident (so no donation; the per-call output buffer is
    # deleted explicitly after download to avoid device-memory churn).
    zeros_dev = jax.device_put(
        np.zeros((N_CORES * B_PER, C, HW), np.float32), sharding)

    def run(x, wc):
        # globals sharded on axis 0: x (16,C,HW), wc tiled to (8*C,).
        xd = jax.device_put(x, sharding)
        outs = jitted(xd, np.tile(wc, N_CORES), zeros_dev)
        res = np.asarray(outs[0])
        outs[0].delete()
        xd.delete()
        return res

    _NC_CACHE["run"] = run
    return run


def _run(x, w1d, trace=False):
    x = np.ascontiguousarray(np.asarray(x, dtype=np.float32)).reshape(B, C, HW)
    # Fold the mean's 1/HW into the center-tap weight: HW is a power of two,
    # so w/HW is exact and sum*(w/HW) rounds identically to (sum/HW)*w.
    wc = np.ascontiguousarray(
        np.asarray(w1d, dtype=np.float32)[:, K_CENTER] / float(HW))
    try:
        out = _get_runner()(x, wc)
    except Exception:
        nc = _get_nc()
        in_maps = [{"x": x[i * B_PER:(i + 1) * B_PER], "wc": wc}
                   for i in range(N_CORES)]
        res = run_bass_kernel_spmd(nc, in_maps, list(range(N_CORES)),
                                   trace=trace)
        out = np.concatenate([res.results[i]["out"]
                              for i in range(N_CORES)], axis=0)
    return out.reshape(B, C, H, W), None


def kernel(x, w1x1=None, b1x1=None, w1d=None):
    out, _ = _run(x, w1d)
    return out

